# revision 1
# baseline (speedup 1.0000x reference)
"""Trainium2 Bass kernel for nn_Expansion (e3nn-style tensor-product expansion).

Math reformulation (verified against the jax reference to ~1e-6):
  h   = silu(node_emb @ lw1 + lb1)                         [B,64]
  hb  = silu(node_emb @ bw1 + bb1)                         [B,64]
  x0  = feat[:,:128] @ W0 / sqrt(128)                      [B,16]
  x1k = feat[:,128+k::3] @ W1 / 8          (k=0,1,2)       [B,16]

The per-sample path contractions  r = sum_w w_path[b,w,:] * x[b,w]  with
w_path = (h @ lw2 + lb2) sliced, are a batched bilinear form

  r[b,p] = sum_{c,w} h'[b,c] x[b,w] M[(c,w), p],   h' = [h, 1]

which becomes a plain matmul over the outer product  z[b,(c,w)] = h'[b,c]*x[b,w]
(K = 65*16 = 1040) against reshaped weight matrices M built from lw2/lb2 on the
host.  This avoids materializing w = h@lw2 ([B,36864], ~600 MB) entirely.

Sharding: pure data parallel, batch 4096 -> 8 cores x 512.  Weights replicated.

Device layout per core (B_c = 512):
  - Inputs are passed pre-transposed ([feat_cols, B_c]) so the contraction dim
    lands on SBUF partitions with no on-device transposes.
  - z is built as 8 K-chunk tiles [128, 512] per z-type (4 types: x0, x1k) via
    DVE multiplies of partition-replicated h' and x tiles; the replication runs
    on the TensorEngine against constant 0/1 selection matrices (Gsel/Tsel),
    landing in SBUF as bf16 via ScalarEngine copies.
  - Main matmuls: out[b_tile=128, N<=512] accumulated over 9 K-chunks (+ one
    65-row bias-MLP chunk for the blk00/blk11 banks) into PSUM.
  - PSUM blocks are copied into an assembled [128, 80*80] SBUF tile with
    strided APs (the 1o interleave), then DMA'd contiguously to HBM.
All path normalization constants are folded into the host-side weight prep.
"""

import sys

import numpy as np

sys.path.insert(0, "/opt/trn_rl_repo")

import ml_dtypes  # noqa: E402

B_TOTAL = 4096
N_CORES = 8
BC = B_TOTAL // N_CORES  # 512 samples per core
P = 128
NB = BC // P  # 4 b-tiles per core
C3 = 1.0 / np.sqrt(3.0)

# matmul dtype mode: "bf16" | "f32r" | "f32"
MM_MODE = "bf16"

_CACHE = {}


def _np_mm_dtype(mode):
    return ml_dtypes.bfloat16 if mode == "bf16" else np.float32


def _build_program(mode, skip_lb2):
    import concourse.tile as tile
    from concourse import bacc, mybir

    F32 = mybir.dt.float32
    MM = mybir.dt.bfloat16 if mode == "bf16" else mybir.dt.float32
    AF = mybir.ActivationFunctionType

    def mmc(ap):
        # reinterpret f32 operands as float32r at the matmul for the fast path
        if mode == "f32r":
            return ap.bitcast(mybir.dt.float32r)
        return ap

    nc = bacc.Bacc("TRN2", target_bir_lowering=False, debug=False,
                   num_devices=N_CORES)

    t = {}
    t["featT"] = nc.dram_tensor("featT", [320, BC], F32, kind="ExternalInput").ap()
    t["node_embT"] = nc.dram_tensor("node_embT", [P, BC], F32, kind="ExternalInput").ap()
    t["W0"] = nc.dram_tensor("W0", [P, 16], F32, kind="ExternalInput").ap()
    t["W1"] = nc.dram_tensor("W1", [64, 16], F32, kind="ExternalInput").ap()
    t["lw1"] = nc.dram_tensor("lw1", [P, 64], F32, kind="ExternalInput").ap()
    t["bw1"] = nc.dram_tensor("bw1", [P, 64], F32, kind="ExternalInput").ap()
    t["lb1c"] = nc.dram_tensor("lb1c", [64, 1], F32, kind="ExternalInput").ap()
    t["bb1c"] = nc.dram_tensor("bb1c", [64, 1], F32, kind="ExternalInput").ap()
    t["R0"] = nc.dram_tensor("R0", [1040, 1280], MM, kind="ExternalInput").ap()
    t["R1"] = nc.dram_tensor("R1", [1040, 1024], MM, kind="ExternalInput").ap()
    t["BB"] = nc.dram_tensor("BB", [65, 1280], MM, kind="ExternalInput").ap()
    t["Gsel"] = nc.dram_tensor("Gsel", [65, 1024], MM, kind="ExternalInput").ap()
    t["Tsel"] = nc.dram_tensor("Tsel", [16, 128], MM, kind="ExternalInput").ap()
    t["out"] = nc.dram_tensor("out", [BC, 6400], F32, kind="ExternalOutput").ap()

    with tile.TileContext(nc) as tc:
        _emit(tc, t, mode, skip_lb2, mybir, MM, F32, AF, mmc)

    nc.compile()
    return nc


def _emit(tc, t, mode, skip_lb2, mybir, MM, F32, AF, mmc):
    nc = tc.nc
    from contextlib import ExitStack

    with ExitStack() as ctx:
        wpool = ctx.enter_context(tc.tile_pool(name="weights", bufs=1))
        apool = ctx.enter_context(tc.tile_pool(name="acts", bufs=1))
        zpool = ctx.enter_context(tc.tile_pool(name="z", bufs=1))
        opool = ctx.enter_context(tc.tile_pool(name="outs", bufs=3))
        pre_psum = ctx.enter_context(tc.tile_pool(name="pre_psum", bufs=1, space="PSUM"))
        prex_psum = ctx.enter_context(tc.tile_pool(name="prex_psum", bufs=2, space="PSUM"))
        main_psum = ctx.enter_context(tc.tile_pool(name="main_psum", bufs=5, space="PSUM"))

        # ---- weights / inputs to SBUF ----
        # small, latency-critical inputs first (they gate the prep chain)
        R0_sb = wpool.tile([P, 9, 1280], MM, tag="R0")
        R1_sb = wpool.tile([P, 9, 1024], MM, tag="R1")
        BB_sb = wpool.tile([65, 1280], MM, tag="BB")
        W0_sb = wpool.tile([P, 16], F32, tag="W0")
        W1_sb = wpool.tile([64, 16], F32, tag="W1")
        lw1_sb = wpool.tile([P, 64], F32, tag="lw1")
        bw1_sb = wpool.tile([P, 64], F32, tag="bw1")
        lb1_sb = wpool.tile([64, 1], F32, tag="lb1")
        bb1_sb = wpool.tile([64, 1], F32, tag="bb1")
        G_sb = wpool.tile([65, 1024], MM, tag="Gsel")
        T_sb = wpool.tile([16, 128], MM, tag="Tsel")

        feats_sb = apool.tile([P, BC], F32, tag="feats")
        featv_sb = [apool.tile([64, BC], F32, name=f"featv{k}", tag=f"featv{k}")
                    for k in range(3)]
        emb_sb = apool.tile([P, BC], F32, tag="emb")
        nc.sync.dma_start(emb_sb[:], t["node_embT"][:])
        nc.sync.dma_start(feats_sb[:], t["featT"][0:128])
        for k in range(3):
            nc.sync.dma_start(featv_sb[k][:], t["featT"][128 + 64 * k:192 + 64 * k])
        nc.sync.dma_start(lw1_sb[:], t["lw1"][:])
        nc.sync.dma_start(bw1_sb[:], t["bw1"][:])
        nc.sync.dma_start(W0_sb[:], t["W0"][:])
        nc.sync.dma_start(W1_sb[:], t["W1"][:])
        nc.sync.dma_start(lb1_sb[:], t["lb1c"][:])
        nc.sync.dma_start(bb1_sb[:], t["bb1c"][:])
        nc.sync.dma_start(G_sb[:], t["Gsel"][:])
        nc.sync.dma_start(T_sb[:], t["Tsel"][:])
        nc.sync.dma_start(BB_sb[:], t["BB"][:])

        # big weight matrices, split by the column blocks the matmul banks
        # consume, so the first banks can start before the full load lands
        r0v = t["R0"][0:1024].rearrange("(q p) n -> p q n", p=P)
        r1v = t["R1"][0:1024].rearrange("(q p) n -> p q n", p=P)
        for c0, c1 in ((0, 512), (512, 1024), (1024, 1280)):
            nc.sync.dma_start(R0_sb[:, 0:8, c0:c1], r0v[:, :, c0:c1])
        for c0, c1 in ((0, 512), (512, 1024)):
            nc.sync.dma_start(R1_sb[:, 0:8, c0:c1], r1v[:, :, c0:c1])
        if not skip_lb2:
            nc.sync.dma_start(R0_sb[0:16, 8, :], t["R0"][1024:1040])
            nc.sync.dma_start(R1_sb[0:16, 8, :], t["R1"][1024:1040])

        # ---- tiny MLP heads: hT, hbT, x0T, x1kT (all [*, BC] with contraction
        #      on partitions) ----
        ph = pre_psum.tile([64, BC], F32, tag="pre")
        nc.tensor.matmul(ph[:], lhsT=lw1_sb[:], rhs=emb_sb[:], start=True, stop=True)
        hp_sb = apool.tile([65, BC], MM, tag="hp")
        nc.scalar.activation(hp_sb[0:64, :], ph[:], AF.Silu, bias=lb1_sb[:])
        nc.any.memset(hp_sb[64:65, :], 1.0)

        pb = pre_psum.tile([64, BC], F32, tag="pre")
        nc.tensor.matmul(pb[:], lhsT=bw1_sb[:], rhs=emb_sb[:], start=True, stop=True)
        hbp_sb = apool.tile([65, BC], MM, tag="hbp")
        nc.scalar.activation(hbp_sb[0:64, :], pb[:], AF.Silu, bias=bb1_sb[:])
        nc.any.memset(hbp_sb[64:65, :], 1.0)

        xs_sb = []
        for tdx in range(4):
            px = prex_psum.tile([16, BC], F32, tag="px")
            if tdx == 0:
                nc.tensor.matmul(px[:], lhsT=W0_sb[:], rhs=feats_sb[:],
                                 start=True, stop=True)
            else:
                nc.tensor.matmul(px[:], lhsT=W1_sb[:], rhs=featv_sb[tdx - 1][:],
                                 start=True, stop=True)
            xf = apool.tile([16, BC], MM, name=f"xf{tdx}", tag=f"xf{tdx}")
            nc.scalar.copy(xf[:], px[:])
            xs_sb.append(xf)
        xs_mm = xs_sb

        # ---- partition-replicated tiles for the z outer product ----
        # Both replications run on PE against constant selection matrices,
        # then land in SBUF (as MM dtype) via ACT copies:
        #   xbc[t][p, b] = x_t[p % 16, b]        (Tsel[w, m] = [m%16 == w])
        #   hbc[q][p, b] = h'[8q + p//16, b]     (Gsel[c, 128q+16c8+w] = [c==8q+c8])
        xbc = []
        for tdx in range(4):
            px_bc = prex_psum.tile([P, BC], F32, name=f"pxbc{tdx}", tag="px")
            nc.tensor.matmul(px_bc[:], lhsT=T_sb[:], rhs=xs_sb[tdx][:],
                             start=True, stop=True)
            xb = apool.tile([P, BC], MM, name=f"xbc{tdx}", tag=f"xbc{tdx}")
            nc.scalar.copy(xb[:], px_bc[:])
            xbc.append(xb)
        hbc = []
        for q in range(8):
            ph_bc = prex_psum.tile([P, BC], F32, name=f"phbc{q}", tag="px")
            nc.tensor.matmul(ph_bc[:], lhsT=G_sb[:, P * q:P * (q + 1)],
                             rhs=hp_sb[:], start=True, stop=True)
            hb = apool.tile([P, BC], MM, name=f"hbc{q}", tag=f"hbc{q}")
            nc.scalar.copy(hb[:], ph_bc[:])
            hbc.append(hb)
        # z[t][q][(c8,w), b] = h'[8q+c8, b] * x_t[w, b]   (MM x MM -> MM on DVE)
        # z-type-outer order matches the PSUM-bank consumption order below, so
        # the first accumulation group unblocks after 8 DVE ops, not 29.
        z = [[None] * 8 for _ in range(4)]
        for tdx in range(4):
            for q in range(8):
                zt = zpool.tile([P, BC], MM, name=f"z{tdx}_{q}", tag=f"z{tdx}_{q}")
                nc.vector.tensor_mul(out=zt[:], in0=hbc[q][:], in1=xbc[tdx][:])
                z[tdx][q] = zt

        # ---- main matmuls + output assembly ----
        def accum2(tdx, rhs_sb, col0, ncols, bias_cols, bsl, psum_ap):
            nmm = 8 + (0 if skip_lb2 else 1) + (1 if bias_cols is not None else 0)
            idx = 0
            for q in range(8):
                idx += 1
                nc.tensor.matmul(psum_ap,
                                 lhsT=mmc(z[tdx][q][:, bsl]),
                                 rhs=mmc(rhs_sb[:, q, col0:col0 + ncols]),
                                 start=(idx == 1), stop=(idx == nmm))
            if not skip_lb2:
                idx += 1
                nc.tensor.matmul(psum_ap,
                                 lhsT=mmc(xs_mm[tdx][:, bsl]),
                                 rhs=mmc(rhs_sb[0:16, 8, col0:col0 + ncols]),
                                 start=False, stop=(idx == nmm))
            if bias_cols is not None:
                idx += 1
                nc.tensor.matmul(psum_ap,
                                 lhsT=mmc(hbp_sb[:, bsl]),
                                 rhs=mmc(BB_sb[:, bias_cols[0]:bias_cols[1]]),
                                 start=False, stop=(idx == nmm))

        for j in range(NB):
            bsl = slice(P * j, P * (j + 1))
            out_t = opool.tile([P, 6400], F32, name="out_t", tag="out_t")
            o3 = out_t.rearrange("p (r c) -> p r c", c=80)          # [128,80,80]
            top = o3[:, 0:32, :]                                     # [128,32,80]
            bot = out_t[:, 2560:6400].rearrange(
                "p (u i c) -> p u i c", i=3, c=80)                   # [128,16,3,80]

            # blk11 off-diagonal zeros
            nc.gpsimd.memset(o3[:, 32:80, 32:80], 0.0)

            # r00 -> blk00 (rows 0..31, cols 0..31), scale folded on host
            p00a = main_psum.tile([P, 512], F32, name="p00a", tag="mp")
            accum2(0, R0_sb, 0, 512, (0, 512), bsl, p00a[:])
            nc.scalar.copy(o3[:, 0:16, 0:32],
                           p00a[:].rearrange("p (u v) -> p u v", v=32))
            p00b = main_psum.tile([P, 512], F32, name="p00b", tag="mp")
            accum2(0, R0_sb, 512, 512, (512, 1024), bsl, p00b[:])
            nc.scalar.copy(o3[:, 16:32, 0:32],
                           p00b[:].rearrange("p (u v) -> p u v", v=32))

            # r11 -> blk11 diagonal-in-(i,j): out[32+3u+i, 32+3v+i]
            p11 = main_psum.tile([P, 512], F32, name="p11", tag="mp")
            accum2(0, R0_sb, 1024, 256, (1024, 1280), bsl, p11[:, 0:256])
            src11 = p11[:, 0:256].rearrange("p (u v) -> p u v", v=16)
            for i in range(3):
                dst = bot[:, :, i, 32:80].rearrange(
                    "p u (v jj) -> p u v jj", jj=3)[:, :, :, i]      # [128,16,16]
                nc.vector.tensor_copy(out=dst, in_=src11)

            # r01k -> blk01: out[u, 32+3v+k], u<32, v<16
            for k in range(3):
                p01 = main_psum.tile([P, 512], F32, name=f"p01_{k}", tag="mp")
                accum2(1 + k, R1_sb, 0, 512, None, bsl, p01[:])
                dst = top[:, :, 32:80].rearrange(
                    "p u (v jj) -> p u v jj", jj=3)[:, :, :, k]      # [128,32,16]
                src = p01[:].rearrange("p (u v) -> p u v", v=16)
                if k == 0:
                    nc.scalar.copy(dst, src)
                else:
                    nc.vector.tensor_copy(out=dst, in_=src)

            # r10i -> blk10: out[32+3u+i, v], u<16, v<32
            for i in range(3):
                p10 = main_psum.tile([P, 512], F32, name=f"p10_{i}", tag="mp")
                accum2(1 + i, R1_sb, 512, 512, None, bsl, p10[:])
                dst = bot[:, :, i, 0:32]                             # [128,16,32]
                src = p10[:].rearrange("p (u v) -> p u v", v=32)
                if i == 0:
                    nc.scalar.copy(dst, src)
                else:
                    nc.vector.tensor_copy(out=dst, in_=src)

            # split the writeback so the top half (blk00|blk01) can leave
            # while the bottom half (blk10|blk11) is still being assembled
            nc.sync.dma_start(t["out"][bsl, 0:2560], out_t[:, 0:2560])
            nc.sync.dma_start(t["out"][bsl, 2560:6400], out_t[:, 2560:6400])


def _prepare(inputs, mode):
    f32 = np.float32
    feat = np.ascontiguousarray(np.asarray(inputs["feat"], dtype=f32))
    node_emb = np.ascontiguousarray(np.asarray(inputs["node_emb"], dtype=f32))
    W0 = np.asarray(inputs["W0"], f32)
    W1 = np.asarray(inputs["W1"], f32)
    lw1 = np.asarray(inputs["lw1"], f32)
    lb1 = np.asarray(inputs["lb1"], f32)
    lw2 = np.asarray(inputs["lw2"], f32)
    lb2 = np.asarray(inputs["lb2"], f32)
    bw1 = np.asarray(inputs["bw1"], f32)
    bb1 = np.asarray(inputs["bb1"], f32)
    bw2 = np.asarray(inputs["bw2"], f32)
    bb2 = np.asarray(inputs["bb2"], f32)

    mmnp = _np_mm_dtype(mode)
    s16 = np.float32(1.0 / 16.0)
    sC = np.float32(C3 / 16.0)

    lw2p = np.concatenate([lw2, lb2[None]], axis=0)           # [65, 36864]
    M00 = lw2p[:, :16384].reshape(1040, 1024) * s16
    M11 = lw2p[:, 16384:20480].reshape(1040, 256) * sC
    M01 = lw2p[:, 20480:28672].reshape(1040, 512) * sC
    M10 = lw2p[:, 28672:36864].reshape(1040, 512) * sC
    R0 = np.ascontiguousarray(np.concatenate([M00, M11], axis=1)).astype(mmnp)
    R1 = np.ascontiguousarray(np.concatenate([M01, M10], axis=1)).astype(mmnp)
    BBf = np.concatenate([bw2, bb2[None]], axis=0)            # [65, 1280]
    BB = np.ascontiguousarray(
        np.concatenate([BBf[:, :1024] * s16, BBf[:, 1024:] * sC], axis=1)
    ).astype(mmnp)

    W0s = np.ascontiguousarray(W0 * np.float32(1.0 / np.sqrt(128.0)))
    W1s = np.ascontiguousarray(W1 * np.float32(1.0 / 8.0))
    lb1c = np.ascontiguousarray(lb1[:, None])
    bb1c = np.ascontiguousarray(bb1[:, None])

    # selection matrices for the PE-based partition replications
    Gsel = np.zeros((65, 1024), np.float32)
    for q in range(8):
        for c8 in range(8):
            Gsel[8 * q + c8, 128 * q + 16 * c8:128 * q + 16 * (c8 + 1)] = 1.0
    Tsel = np.zeros((16, 128), np.float32)
    for w in range(16):
        Tsel[w, w::16] = 1.0
    Gsel = Gsel.astype(mmnp)
    Tsel = Tsel.astype(mmnp)

    skip_lb2 = not bool(np.any(lb2))

    in_maps = []
    for i in range(N_CORES):
        sl = slice(i * BC, (i + 1) * BC)
        fs = feat[sl]
        featT = np.ascontiguousarray(
            np.concatenate(
                [fs[:, :128], fs[:, 128::3], fs[:, 129::3], fs[:, 130::3]],
                axis=1).T)                                     # [320, BC]
        embT = np.ascontiguousarray(node_emb[sl].T)            # [128, BC]
        in_maps.append({
            "featT": featT,
            "node_embT": embT,
            "W0": W0s, "W1": W1s,
            "lw1": lw1, "bw1": bw1,
            "lb1c": lb1c, "bb1c": bb1c,
            "R0": R0, "R1": R1, "BB": BB, "Gsel": Gsel, "Tsel": Tsel,
        })
    return in_maps, skip_lb2


def run(inputs, mode=None, trace=False):
    """Build (cached), run on 8 cores, gather. Returns (out, results)."""
    mode = mode or MM_MODE
    in_maps, skip_lb2 = _prepare(inputs, mode)
    key = (mode, skip_lb2)
    if key not in _CACHE:
        _CACHE[key] = _build_program(mode, skip_lb2)
    nc = _CACHE[key]

    from concourse.bass_utils import run_bass_kernel_spmd
    res = run_bass_kernel_spmd(nc, in_maps, list(range(N_CORES)), trace=trace)
    out = np.concatenate(
        [res.results[i]["out"].reshape(BC, 80, 80) for i in range(N_CORES)],
        axis=0)
    return out.astype(np.float32), res


def kernel(**inputs):
    out, _ = run(inputs)
    return out



# revision 3
# speedup vs baseline: 1.1713x; 1.1713x over previous
"""Trainium2 Bass kernel for nn_Expansion (e3nn-style tensor-product expansion).

Math reformulation (verified against the jax reference):
  h   = silu(node_emb @ lw1 + lb1)                         [B,64]
  hb  = silu(node_emb @ bw1 + bb1)                         [B,64]
  x0  = feat[:,:128] @ W0 / sqrt(128)                      [B,16]
  x1k = feat[:,128+k::3] @ W1 / 8          (k=0,1,2)       [B,16]

The per-sample path contractions with wpath = (h @ lw2 + lb2) are a batched
bilinear form

  r[b,p] = sum_{c,w} h'[b,c] x[b,w] M[(c,w), p],   h' = [h, 1]

computed as a plain matmul over the outer product z[b,(c,w)] = h'[b,c]*x[b,w]
(K = 64*16 = 1024 in 8 chunks of 128, + a K=65 chunk for the bias MLP @ BB)
against reshaped weight matrices M built from lw2/bw2 on the host.  This
avoids materializing w = h@lw2 ([B,36864], ~600 MB) entirely.

v2 layout decisions (all driven by the instruction-cost timeline model):
  - The partition-replicated tiles hbc[q][(c8,w),b] = h[8q+c8,b] and
    xbc[t][(c8,w),b] = x_t[w,b] are produced DIRECTLY by the MLP/projection
    matmuls using host-replicated weight columns (lw1rep/W0rep/W1rep), so no
    separate selection-matrix matmuls or extra copies exist.  ACT applies
    silu (with replicated bias) straight from PSUM into bf16 SBUF tiles.
  - All matmuls run in bf16 (fp32 matmuls cost 4x); inputs ship as bf16.
  - The device output is the raw concatenation of the 9 PSUM bank groups per
    sample, [BC, 4352] bf16 — a fixed column permutation of the nonzero
    entries of the [80,80] block matrix.  The host scatters it into the
    final [B,80,80] float32 (incl. the blk11 diagonal triplication and the
    structural zeros), so the device does zero strided/duplicated writes.
  - First b-tile interleaves its three z0-consuming bank groups q-major so
    the PE consumption rate (3 matmuls / z0 chunk) matches ACT's silu
    production rate during warmup.

Sharding: pure data parallel, batch 4096 -> 8 cores x 512.  Weights
replicated; no cross-device communication.
"""

import sys

import numpy as np

sys.path.insert(0, "/opt/trn_rl_repo")

import ml_dtypes  # noqa: E402

B_TOTAL = 4096
N_CORES = 8
BC = B_TOTAL // N_CORES  # 512 samples per core
P = 128
NB = BC // P  # 4 b-tiles per core
C3 = 1.0 / np.sqrt(3.0)
NCOL = 4352  # packed device output columns per sample

# matmul dtype mode: "bf16" | "f32"
MM_MODE = "bf16"

_CACHE = {}


def _np_mm_dtype(mode):
    return ml_dtypes.bfloat16 if mode == "bf16" else np.float32


def _build_program(mode, skip_lb2):
    import concourse.tile as tile
    from concourse import bacc, mybir

    F32 = mybir.dt.float32
    MM = mybir.dt.bfloat16 if mode == "bf16" else mybir.dt.float32
    AF = mybir.ActivationFunctionType

    nc = bacc.Bacc("TRN2", target_bir_lowering=False, debug=False,
                   num_devices=N_CORES)

    t = {}
    t["featT"] = nc.dram_tensor("featT", [320, BC], MM, kind="ExternalInput").ap()
    t["embT"] = nc.dram_tensor("embT", [P, BC], MM, kind="ExternalInput").ap()
    t["wpk"] = nc.dram_tensor("wpk", [P, 1344], MM, kind="ExternalInput").ap()
    t["spk"] = nc.dram_tensor("spk", [P, 9], F32, kind="ExternalInput").ap()
    t["BBp"] = nc.dram_tensor("BBp", [65, 1280], MM, kind="ExternalInput").ap()
    t["R0"] = nc.dram_tensor("R0", [1024, 1280], MM, kind="ExternalInput").ap()
    t["R1"] = nc.dram_tensor("R1", [1024, 1024], MM, kind="ExternalInput").ap()
    if not skip_lb2:
        t["R0x"] = nc.dram_tensor("R0x", [16, 1280], MM, kind="ExternalInput").ap()
        t["R1x"] = nc.dram_tensor("R1x", [16, 1024], MM, kind="ExternalInput").ap()
    t["out"] = nc.dram_tensor("out", [BC, NCOL], MM, kind="ExternalOutput").ap()

    with tile.TileContext(nc) as tc:
        _emit(tc, t, mode, skip_lb2, mybir, MM, F32, AF)

    nc.compile()
    return nc


def _emit(tc, t, mode, skip_lb2, mybir, MM, F32, AF):
    nc = tc.nc
    from contextlib import ExitStack

    with ExitStack() as ctx:
        wpool = ctx.enter_context(tc.tile_pool(name="weights", bufs=1))
        apool = ctx.enter_context(tc.tile_pool(name="acts", bufs=1))
        zpool = ctx.enter_context(tc.tile_pool(name="z", bufs=1))
        opool = ctx.enter_context(tc.tile_pool(name="outs", bufs=3))
        prep_psum = ctx.enter_context(tc.tile_pool(name="prep_psum", bufs=3, space="PSUM"))
        main_psum = ctx.enter_context(tc.tile_pool(name="main_psum", bufs=5, space="PSUM"))

        # ---- SBUF tiles ----
        wpk_sb = wpool.tile([P, 1344], MM, tag="wpk")
        spk_sb = wpool.tile([P, 9], F32, tag="spk")
        BB_sb = wpool.tile([65, 1280], MM, tag="BBp")
        R0_sb = wpool.tile([P, 8, 1280], MM, tag="R0")
        R1_sb = wpool.tile([P, 8, 1024], MM, tag="R1")
        if not skip_lb2:
            R0x_sb = wpool.tile([16, 1280], MM, tag="R0x")
            R1x_sb = wpool.tile([16, 1024], MM, tag="R1x")

        emb_sb = apool.tile([P, BC], MM, tag="emb")
        feats_sb = apool.tile([P, BC], MM, tag="feats")
        featv_sb = [apool.tile([64, BC], MM, name=f"featv{k}", tag=f"featv{k}")
                    for k in range(3)]
        hbp_sb = apool.tile([65, BC], MM, tag="hbp")
        hbc = [apool.tile([P, BC], MM, name=f"hbc{q}", tag=f"hbc{q}")
               for q in range(8)]
        xbc = [apool.tile([P, BC], MM, name=f"xbc{t_}", tag=f"xbc{t_}")
               for t_ in range(4)]

        # ---- input DMAs, ordered by first consumer ----
        nc.sync.dma_start(emb_sb[:], t["embT"][:])
        nc.sync.dma_start(wpk_sb[:, 1024:1344], t["wpk"][:, 1024:1344])
        nc.sync.dma_start(feats_sb[:], t["featT"][0:128])
        nc.sync.dma_start(spk_sb[:], t["spk"][:])
        nc.sync.dma_start(wpk_sb[:, 0:1024], t["wpk"][:, 0:1024])
        nc.sync.dma_start(BB_sb[:], t["BBp"][:])
        for k in range(3):
            nc.sync.dma_start(featv_sb[k][:], t["featT"][128 + 64 * k:192 + 64 * k])
        # R0 arrives q-chunk-major to match the phase-1 q-major consumption;
        # R1 follows in halves (phase 2/3 consume it much later)
        r0v = t["R0"].rearrange("(q p) n -> p q n", p=P)
        r1v = t["R1"].rearrange("(q p) n -> p q n", p=P)
        for q in range(8):
            nc.sync.dma_start(R0_sb[:, q, :], r0v[:, q, :])
        nc.sync.dma_start(R1_sb[:, 0:4, :], r1v[:, 0:4, :])
        nc.sync.dma_start(R1_sb[:, 4:8, :], r1v[:, 4:8, :])
        if not skip_lb2:
            nc.sync.dma_start(R0x_sb[:], t["R0x"][:])
            nc.sync.dma_start(R1x_sb[:], t["R1x"][:])

        # ---- PE warmup: dummy matmuls on a memset tile keep the PE busy
        # (and its p-state ramping) while the first input DMAs land ----
        warm_sb = apool.tile([P, P], MM, tag="warm")
        nc.vector.memset(warm_sb[:], 0.0)
        pwarm = prep_psum.tile([P, P], F32, tag="pp")
        for _ in range(26):
            nc.tensor.matmul(pwarm[:], lhsT=warm_sb[:], rhs=warm_sb[:],
                             start=True, stop=True)

        # ---- prep: bias-MLP head + replicated-form h and x tiles ----
        # hbp[c,b] = silu(bw1^T emb + bb1), plus a ones row for the bb2 path
        ph = prep_psum.tile([64, BC], F32, tag="pp")
        nc.tensor.matmul(ph[:], lhsT=wpk_sb[:, 1280:1344], rhs=emb_sb[:],
                         start=True, stop=True)
        nc.scalar.activation(hbp_sb[0:64, :], ph[:], AF.Silu,
                             bias=spk_sb[0:64, 8:9])
        nc.gpsimd.memset(hbp_sb[64:65, :], 1.0)

        # xbc[0][(c8,w),b] = x0[w,b] via column-replicated W0 (copy on DVE —
        # ACT is saturated by the silu chain during prep)
        px0 = prep_psum.tile([P, BC], F32, tag="pp")
        nc.tensor.matmul(px0[:], lhsT=wpk_sb[:, 1024:1152], rhs=feats_sb[:],
                         start=True, stop=True)
        nc.vector.tensor_copy(out=xbc[0][:], in_=px0[:])

        # hbc[q][(c8,w),b] = silu((lw1rep_q)^T emb + lb1rep_q) = h[8q+c8,b]
        # z0 muls interleave so the first bank groups can start consuming
        z = [[None] * 8 for _ in range(4)]
        for q in range(8):
            phq = prep_psum.tile([P, BC], F32, name=f"ph{q}", tag="pp")
            nc.tensor.matmul(phq[:], lhsT=wpk_sb[:, P * q:P * (q + 1)],
                             rhs=emb_sb[:], start=True, stop=True)
            nc.scalar.activation(hbc[q][:], phq[:], AF.Silu,
                                 bias=spk_sb[:, q:q + 1])
            zt = zpool.tile([P, BC], MM, name=f"z0_{q}", tag=f"z0_{q}")
            nc.vector.tensor_mul(out=zt[:], in0=hbc[q][:], in1=xbc[0][:])
            z[0][q] = zt

        # xbc[1..3] via column-replicated W1, then their z tiles
        for tdx in range(1, 4):
            pxt = prep_psum.tile([P, BC], F32, name=f"px{tdx}", tag="pp")
            nc.tensor.matmul(pxt[:], lhsT=wpk_sb[0:64, 1152:1280],
                             rhs=featv_sb[tdx - 1][:], start=True, stop=True)
            nc.vector.tensor_copy(out=xbc[tdx][:], in_=pxt[:])
            for q in range(8):
                zt = zpool.tile([P, BC], MM, name=f"z{tdx}_{q}", tag=f"z{tdx}_{q}")
                nc.vector.tensor_mul(out=zt[:], in0=hbc[q][:], in1=xbc[tdx][:])
                z[tdx][q] = zt

        # ---- main accumulation groups, group-type-major ----
        # phase 1 (needs only R0): per b-tile g0..g2 = z0 @ R0 cols
        # (0:512 | 512:1024 | 1024:1280) + the BB bias chunk, q-major
        # interleaved across the three banks.
        # phase 2 (R1[:, 0:512]): g3..g5 = z[1+k] @ R1 left  (blk01)
        # phase 3 (R1[:, 512:1024]): g6..g8 = z[1+i] @ R1 right (blk10)
        def copy_out(eng, dst_ap, src_ap):
            if eng == "a":
                nc.scalar.copy(dst_ap, src_ap)
            else:
                nc.vector.tensor_copy(out=dst_ap, in_=src_ap)

        out_t = [opool.tile([P, NCOL], MM, name=f"out_t{j}", tag=f"out_t{j}")
                 for j in range(NB)]
        P1_ENG = ["add", "add", "add", "add"]  # per-j engines for g0/g1/g2
        P23_ENG = ["adadad", "dadada", "adadad", "dadada"]  # per-j g3..g8

        for j in range(NB):
            bsl = slice(P * j, P * (j + 1))
            pg = [main_psum.tile([P, 512], F32, name=f"pg{j}_{g}", tag="mp")
                  for g in range(3)]
            gsl = [pg[0][:], pg[1][:], pg[2][:, 0:256]]
            gcols = [(0, 512), (512, 1024), (1024, 1280)]
            for g in range(3):
                c0, c1 = gcols[g]
                nc.tensor.matmul(gsl[g], lhsT=hbp_sb[:, bsl],
                                 rhs=BB_sb[:, c0:c1], start=True, stop=False)
            for q in range(8):
                last = skip_lb2 and q == 7
                for g in range(3):
                    c0, c1 = gcols[g]
                    nc.tensor.matmul(gsl[g], lhsT=z[0][q][:, bsl],
                                     rhs=R0_sb[:, q, c0:c1],
                                     start=False, stop=last)
            if not skip_lb2:
                for g in range(3):
                    c0, c1 = gcols[g]
                    nc.tensor.matmul(gsl[g], lhsT=xbc[0][0:16, bsl],
                                     rhs=R0x_sb[:, c0:c1],
                                     start=False, stop=True)
            for g in range(3):
                c0, c1 = gcols[g]
                copy_out(P1_ENG[j][g], out_t[j][:, c0:c1], gsl[g])
            nc.sync.dma_start(t["out"][bsl, 0:1280], out_t[j][:, 0:1280])

        for phase in range(2):
            rc = (0, 512) if phase == 0 else (512, 1024)
            for j in range(NB):
                bsl = slice(P * j, P * (j + 1))
                for mi in range(3):
                    m = 3 * phase + mi
                    tdx = 1 + mi
                    pgm = main_psum.tile([P, 512], F32, name=f"pm{j}_{m}",
                                         tag="mp")
                    for q in range(8):
                        last = skip_lb2 and q == 7
                        nc.tensor.matmul(pgm[:], lhsT=z[tdx][q][:, bsl],
                                         rhs=R1_sb[:, q, rc[0]:rc[1]],
                                         start=(q == 0), stop=last)
                    if not skip_lb2:
                        nc.tensor.matmul(pgm[:], lhsT=xbc[tdx][0:16, bsl],
                                         rhs=R1x_sb[:, rc[0]:rc[1]],
                                         start=False, stop=True)
                    c0 = 1280 + 512 * m
                    copy_out(P23_ENG[j][m], out_t[j][:, c0:c0 + 512], pgm[:])
                c0 = 1280 + 1536 * phase
                last_tile = phase == 1 and j == NB - 1
                bout = t["out"][bsl]
                if last_tile:
                    # fine-grained final writeback to shrink the DMA tail
                    nc.sync.dma_start(bout[:, c0:c0 + 1024],
                                      out_t[j][:, c0:c0 + 1024])
                    nc.sync.dma_start(bout[:, c0 + 1024:c0 + 1536],
                                      out_t[j][:, c0 + 1024:c0 + 1536])
                else:
                    nc.sync.dma_start(bout[:, c0:c0 + 1536],
                                      out_t[j][:, c0:c0 + 1536])


def _prepare(inputs, mode):
    f32 = np.float32
    feat = np.ascontiguousarray(np.asarray(inputs["feat"], dtype=f32))
    node_emb = np.ascontiguousarray(np.asarray(inputs["node_emb"], dtype=f32))
    W0 = np.asarray(inputs["W0"], f32)
    W1 = np.asarray(inputs["W1"], f32)
    lw1 = np.asarray(inputs["lw1"], f32)
    lb1 = np.asarray(inputs["lb1"], f32)
    lw2 = np.asarray(inputs["lw2"], f32)
    lb2 = np.asarray(inputs["lb2"], f32)
    bw1 = np.asarray(inputs["bw1"], f32)
    bb1 = np.asarray(inputs["bb1"], f32)
    bw2 = np.asarray(inputs["bw2"], f32)
    bb2 = np.asarray(inputs["bb2"], f32)

    mmnp = _np_mm_dtype(mode)
    s16 = np.float32(1.0 / 16.0)
    sC = np.float32(C3 / 16.0)

    # weight matrices for the main contraction, path scales folded in
    lw2p = np.concatenate([lw2, lb2[None]], axis=0)           # [65, 36864]
    M00 = lw2p[:, :16384].reshape(1040, 1024) * s16
    M11 = lw2p[:, 16384:20480].reshape(1040, 256) * sC
    M01 = lw2p[:, 20480:28672].reshape(1040, 512) * sC
    M10 = lw2p[:, 28672:36864].reshape(1040, 512) * sC
    R0f = np.concatenate([M00, M11], axis=1)                  # [1040, 1280]
    R1f = np.concatenate([M01, M10], axis=1)                  # [1040, 1024]
    R0 = np.ascontiguousarray(R0f[0:1024]).astype(mmnp)
    R1 = np.ascontiguousarray(R1f[0:1024]).astype(mmnp)
    R0x = np.ascontiguousarray(R0f[1024:1040]).astype(mmnp)
    R1x = np.ascontiguousarray(R1f[1024:1040]).astype(mmnp)
    BBf = np.concatenate([bw2, bb2[None]], axis=0)            # [65, 1280]
    BBp = np.ascontiguousarray(
        np.concatenate([BBf[:, :1024] * s16, BBf[:, 1024:] * sC], axis=1)
    ).astype(mmnp)

    # replicated-column weights: output partition (c8,w) = 16*c8 + w
    W0s = W0 * np.float32(1.0 / np.sqrt(128.0))               # [128, 16]
    W1s = W1 * np.float32(1.0 / 8.0)                          # [64, 16]
    rep = np.arange(1024)
    gsel = (rep // 128) * 8 + (rep % 128) // 16               # c = 8q + c8
    lw1rep = lw1[:, gsel]                                     # [128, 1024]
    W0rep = np.tile(W0s, (1, 8))                              # [128, 128]
    W1rep = np.tile(W1s, (1, 8))                              # [64, 128]
    wpk = np.zeros((128, 1344), f32)
    wpk[:, 0:1024] = lw1rep
    wpk[:, 1024:1152] = W0rep
    wpk[0:64, 1152:1280] = W1rep
    wpk[:, 1280:1344] = bw1
    wpk = np.ascontiguousarray(wpk).astype(mmnp)

    spk = np.zeros((128, 9), f32)
    for q in range(8):
        spk[:, q] = lb1[8 * q + np.arange(128) // 16]
    spk[0:64, 8] = bb1
    spk = np.ascontiguousarray(spk)

    skip_lb2 = not bool(np.any(lb2))

    in_maps = []
    for i in range(N_CORES):
        sl = slice(i * BC, (i + 1) * BC)
        fs = feat[sl]
        featT = np.ascontiguousarray(
            np.concatenate(
                [fs[:, :128], fs[:, 128::3], fs[:, 129::3], fs[:, 130::3]],
                axis=1).T.astype(mmnp))                       # [320, BC]
        embT = np.ascontiguousarray(node_emb[sl].T.astype(mmnp))  # [128, BC]
        m = {
            "featT": featT,
            "embT": embT,
            "wpk": wpk, "spk": spk, "BBp": BBp,
            "R0": R0, "R1": R1,
        }
        if not skip_lb2:
            m["R0x"] = R0x
            m["R1x"] = R1x
        in_maps.append(m)
    return in_maps, skip_lb2


def _unpack_output(buf):
    """[B, 4352] packed columns -> [B, 80, 80] float32."""
    bf = buf.astype(np.float32)
    n = bf.shape[0]
    out3 = np.zeros((n, 80, 80), np.float32)
    out3[:, 0:16, 0:32] = bf[:, 0:512].reshape(n, 16, 32)
    out3[:, 16:32, 0:32] = bf[:, 512:1024].reshape(n, 16, 32)
    p11 = bf[:, 1024:1280].reshape(n, 16, 16)
    for i in range(3):
        out3[:, 32 + i::3, 32 + i::3] = p11
    for k in range(3):
        out3[:, 0:32, 32 + k::3] = \
            bf[:, 1280 + 512 * k:1792 + 512 * k].reshape(n, 32, 16)
    for i in range(3):
        out3[:, 32 + i::3, 0:32] = \
            bf[:, 2816 + 512 * i:3328 + 512 * i].reshape(n, 16, 32)
    return out3


def run(inputs, mode=None, trace=False):
    """Build (cached), run on 8 cores, gather. Returns (out, results)."""
    mode = mode or MM_MODE
    in_maps, skip_lb2 = _prepare(inputs, mode)
    key = (mode, skip_lb2)
    if key not in _CACHE:
        _CACHE[key] = _build_program(mode, skip_lb2)
    nc = _CACHE[key]

    from concourse.bass_utils import run_bass_kernel_spmd
    res = run_bass_kernel_spmd(nc, in_maps, list(range(N_CORES)), trace=trace)
    buf = np.concatenate([res.results[i]["out"] for i in range(N_CORES)], axis=0)
    return _unpack_output(buf), res


def kernel(**inputs):
    out, _ = run(inputs)
    return out


# revision 35
# speedup vs baseline: 1.2883x; 1.0999x over previous
"""Trainium2 Bass kernel for nn_Expansion (e3nn-style tensor-product expansion).

Math reformulation (verified against the jax reference):
  h   = silu(node_emb @ lw1 + lb1)                         [B,64]
  hb  = silu(node_emb @ bw1 + bb1)                         [B,64]
  x0  = feat[:,:128] @ W0 / sqrt(128)                      [B,16]
  x1k = feat[:,128+k::3] @ W1 / 8          (k=0,1,2)       [B,16]

The per-sample path contractions with wpath = (h @ lw2 + lb2) are a batched
bilinear form

  r[b,p] = sum_{c,w} h'[b,c] x[b,w] M[(c,w), p],   h' = [h, 1]

computed as a plain matmul over the outer product z[b,(c,w)] = h'[b,c]*x[b,w]
(K = 64*16 = 1024 in 8 chunks of 128, + a K=65 chunk for the bias MLP @ BB)
against reshaped weight matrices M built from lw2/bw2 on the host.  This
avoids materializing w = h@lw2 ([B,36864], ~600 MB) entirely.

v2 layout decisions (all driven by the instruction-cost timeline model):
  - The partition-replicated tiles hbc[q][(c8,w),b] = h[8q+c8,b] and
    xbc[t][(c8,w),b] = x_t[w,b] are produced DIRECTLY by the MLP/projection
    matmuls using host-replicated weight columns (lw1rep/W0rep/W1rep), so no
    separate selection-matrix matmuls or extra copies exist.  ACT applies
    silu (with replicated bias) straight from PSUM into bf16 SBUF tiles.
  - All matmuls run in bf16 (fp32 matmuls cost 4x); inputs ship as bf16.
  - The device output is the raw concatenation of the 9 PSUM bank groups per
    sample, [BC, 4352] bf16 — a fixed column permutation of the nonzero
    entries of the [80,80] block matrix.  The host scatters it into the
    final [B,80,80] float32 (incl. the blk11 diagonal triplication and the
    structural zeros), so the device does zero strided/duplicated writes.
  - First b-tile interleaves its three z0-consuming bank groups q-major so
    the PE consumption rate (3 matmuls / z0 chunk) matches ACT's silu
    production rate during warmup.

Sharding: pure data parallel, batch 4096 -> 8 cores x 512.  Weights
replicated; no cross-device communication.
"""

import sys

import numpy as np

sys.path.insert(0, "/opt/trn_rl_repo")

import ml_dtypes  # noqa: E402

B_TOTAL = 4096
N_CORES = 8
BC = B_TOTAL // N_CORES  # 512 samples per core
P = 128
NB = BC // P  # 4 b-tiles per core
C3 = 1.0 / np.sqrt(3.0)
NCOL = 4352  # packed device output columns per sample

# matmul dtype mode: "bf16" | "f32"
MM_MODE = "bf16"

_CACHE = {}


def _np_mm_dtype(mode):
    return ml_dtypes.bfloat16 if mode == "bf16" else np.float32


def _build_program(mode, skip_lb2):
    import concourse.tile as tile
    from concourse import bacc, mybir

    F32 = mybir.dt.float32
    MM = mybir.dt.bfloat16 if mode == "bf16" else mybir.dt.float32
    AF = mybir.ActivationFunctionType

    nc = bacc.Bacc("TRN2", target_bir_lowering=False, debug=False,
                   num_devices=N_CORES)

    t = {}
    t["featT"] = nc.dram_tensor("featT", [320, BC], MM, kind="ExternalInput").ap()
    t["wpk"] = nc.dram_tensor("wpk", [P, 1872], MM, kind="ExternalInput").ap()
    t["BBp"] = nc.dram_tensor("BBp", [65, 1280], MM, kind="ExternalInput").ap()
    t["R0"] = nc.dram_tensor("R0", [1024, 1280], MM, kind="ExternalInput").ap()
    t["R1"] = nc.dram_tensor("R1", [1024, 1024], MM, kind="ExternalInput").ap()
    if not skip_lb2:
        t["R0x"] = nc.dram_tensor("R0x", [16, 1280], MM, kind="ExternalInput").ap()
        t["R1x"] = nc.dram_tensor("R1x", [16, 1024], MM, kind="ExternalInput").ap()
    t["out"] = nc.dram_tensor("out", [BC, NCOL], MM, kind="ExternalOutput").ap()

    with tile.TileContext(nc) as tc:
        _emit(tc, t, mode, skip_lb2, mybir, MM, F32, AF)

    nc.compile()
    return nc


def _emit(tc, t, mode, skip_lb2, mybir, MM, F32, AF):
    nc = tc.nc
    from contextlib import ExitStack

    with ExitStack() as ctx:
        wpool = ctx.enter_context(tc.tile_pool(name="weights", bufs=1))
        apool = ctx.enter_context(tc.tile_pool(name="acts", bufs=1))
        zpool = ctx.enter_context(tc.tile_pool(name="z", bufs=1))
        opool = ctx.enter_context(tc.tile_pool(name="outs", bufs=3))
        prep_psum = ctx.enter_context(tc.tile_pool(name="prep_psum", bufs=3, space="PSUM"))
        main_psum = ctx.enter_context(tc.tile_pool(name="main_psum", bufs=5, space="PSUM"))

        # ---- SBUF tiles ----
        # wpk column layout: [embT(512) | lw1rep q0 | q1 | lb1rep(8) |
        #   bb1(1) | pad(7) | bw1(64) | W0rep(128) || lw1rep q2..q7 |
        #   W1rep(128, rows 0:64)]
        # The prefix [0:976] is everything the first prep matmuls need.
        wpk_sb = wpool.tile([P, 1872], MM, tag="wpk")
        BB_sb = wpool.tile([65, 1280], MM, tag="BBp")
        R0_sb = wpool.tile([P, 8, 1280], MM, tag="R0")
        R1_sb = wpool.tile([P, 8, 1024], MM, tag="R1")
        if not skip_lb2:
            R0x_sb = wpool.tile([16, 1280], MM, tag="R0x")
            R1x_sb = wpool.tile([16, 1024], MM, tag="R1x")

        feats_sb = apool.tile([P, BC], MM, tag="feats")
        featv_sb = apool.tile([64, 3, BC], MM, tag="featv")
        hbp_sb = apool.tile([65, BC], MM, tag="hbp")
        hbc = [apool.tile([P, BC], MM, name=f"hbc{q}", tag=f"hbc{q}")
               for q in range(8)]
        xbc = [apool.tile([P, BC], MM, name=f"xbc{t_}", tag=f"xbc{t_}")
               for t_ in range(4)]

        # ---- input DMAs, ordered by first consumer ----
        # R0 arrives q-chunk-major to match the phase-1 q-major consumption;
        # R1 follows in quarters (phase 2/3 consume it much later)
        r0v = t["R0"].rearrange("(q p) n -> p q n", p=P)
        r1v = t["R1"].rearrange("(q p) n -> p q n", p=P)
        nc.sync.dma_start(wpk_sb[:, 0:976], t["wpk"][:, 0:976])
        nc.sync.dma_start(feats_sb[:], t["featT"][0:128])
        nc.sync.dma_start(R0_sb[:, 0, :], r0v[:, 0, :])
        nc.sync.dma_start(wpk_sb[:, 976:1872], t["wpk"][:, 976:1872])
        nc.sync.dma_start(R0_sb[:, 1, :], r0v[:, 1, :])
        nc.sync.dma_start(BB_sb[:], t["BBp"][:])
        nc.sync.dma_start(R0_sb[:, 2, :], r0v[:, 2, :])
        nc.sync.dma_start(featv_sb[:],
                          t["featT"][128:320].rearrange("(k p) b -> p k b", k=3))
        for q in range(3, 8):
            nc.sync.dma_start(R0_sb[:, q, :], r0v[:, q, :])
        nc.sync.dma_start(R1_sb[:, 0:4, 0:512], r1v[:, 0:4, 0:512])
        nc.sync.dma_start(R1_sb[:, 4:8, 0:512], r1v[:, 4:8, 0:512])
        nc.sync.dma_start(R1_sb[:, 0:4, 512:1024], r1v[:, 0:4, 512:1024])
        nc.sync.dma_start(R1_sb[:, 4:8, 512:1024], r1v[:, 4:8, 512:1024])
        if not skip_lb2:
            nc.sync.dma_start(R0x_sb[:], t["R0x"][:])
            nc.sync.dma_start(R1x_sb[:], t["R1x"][:])

        # ---- PE warmup: dummy matmuls keep the PE busy (and its p-state
        # ramping) while the first input DMAs land.  The operand tile is
        # never initialized — results land in a scratch PSUM bank that is
        # cleared (start=True) before any real use. ----
        warm_sb = apool.tile([P, P], MM, tag="warm")
        nc.vector.memset(warm_sb[:], 0.0)
        # preload the ACT activation table so the first real silu doesn't pay
        # the ~1.3us table-load latency (separate tile: no dep on warm_sb)
        tbl_sb = apool.tile([1, 4], MM, tag="tbl")
        nc.vector.memset(tbl_sb[:], 0.0)
        nc.scalar.activation(tbl_sb[0:1, 0:1], tbl_sb[0:1, 2:3], AF.Silu)
        pwarm = prep_psum.tile([P, P], F32, tag="pp")
        for _ in range(22):
            nc.tensor.matmul(pwarm[:], lhsT=warm_sb[:], rhs=warm_sb[:],
                             start=True, stop=True)

        # ---- prep emitters ----
        z = [[None] * 8 for _ in range(4)]
        HCOL = [512, 640, 976, 1104, 1232, 1360, 1488, 1616]

        def h_mm(q):
            # hbc[q][(c8,w),b] = silu((lw1rep_q)^T emb + lb1rep_q) = h[8q+c8,b]
            phq = prep_psum.tile([P, BC], F32, name=f"ph{q}", tag="pp")
            nc.tensor.matmul(phq[:], lhsT=wpk_sb[:, HCOL[q]:HCOL[q] + P],
                             rhs=wpk_sb[:, 0:512], start=True, stop=True)
            nc.scalar.activation(hbc[q][:], phq[:], AF.Silu,
                                 bias=wpk_sb[:, 768 + q:769 + q])

        def x_mm(tdx):
            # xbc[t][(c8,w),b] = x_t[w,b] via column-replicated W0/W1
            # (copy on DVE — ACT is saturated by the silu chain during prep)
            pxt = prep_psum.tile([P, BC], F32, name=f"px{tdx}", tag="pp")
            if tdx == 0:
                nc.tensor.matmul(pxt[:], lhsT=wpk_sb[:, 848:976],
                                 rhs=feats_sb[:], start=True, stop=True)
            else:
                nc.tensor.matmul(pxt[:], lhsT=wpk_sb[0:64, 1744:1872],
                                 rhs=featv_sb[:, tdx - 1, :],
                                 start=True, stop=True)
            nc.vector.tensor_copy(out=xbc[tdx][:], in_=pxt[:])

        def z_mul(tdx, q):
            zt = zpool.tile([P, BC], MM, name=f"z{tdx}_{q}", tag=f"z{tdx}_{q}")
            nc.vector.tensor_mul(out=zt[:], in0=hbc[q][:], in1=xbc[tdx][:])
            z[tdx][q] = zt

        h_mm(0)
        h_mm(1)
        x_mm(0)

        # hbp[c,b] = silu(bw1^T emb + bb1), plus a ones row for the bb2 path
        ph = prep_psum.tile([64, BC], F32, tag="pp")
        nc.tensor.matmul(ph[:], lhsT=wpk_sb[:, 784:848], rhs=wpk_sb[:, 0:512],
                         start=True, stop=True)
        nc.scalar.activation(hbp_sb[0:64, :], ph[:], AF.Silu,
                             bias=wpk_sb[0:64, 776:777])
        nc.gpsimd.memset(hbp_sb[64:65, :], 1.0)

        # fillers: cover the silu->z_mul latency before the first z matmul
        for _ in range(10):
            nc.tensor.matmul(pwarm[:], lhsT=warm_sb[:], rhs=warm_sb[:],
                             start=True, stop=True)

        # ---- main accumulation groups, group-type-major ----
        # phase 1 (needs only R0): per b-tile g0..g2 = z0 @ R0 cols
        # (0:512 | 512:1024 | 1024:1280) + the BB bias chunk, q-major
        # interleaved across the three banks.  The j==0 pass interleaves the
        # remaining prep matmuls so the PE tracks ACT's silu cadence.
        # phase 2 (R1[:, 0:512]): g3..g5 = z[1+k] @ R1 left  (blk01)
        # phase 3 (R1[:, 512:1024]): g6..g8 = z[1+i] @ R1 right (blk10)
        def copy_out(eng, dst_ap, src_ap):
            if eng == "a":
                nc.scalar.copy(dst_ap, src_ap)
            else:
                nc.vector.tensor_copy(out=dst_ap, in_=src_ap)

        out_t = [opool.tile([P, NCOL], MM, name=f"out_t{j}", tag=f"out_t{j}")
                 for j in range(NB)]
        P1_ENG = ["aaa", "aaa", "aaa", "ada"]  # per-j engines for g0/g1/g2
        P23_ENG = ["adadad", "dadada", "adadad", "dadada"]  # per-j g3..g8

        def phase1(j, final):
            bsl = slice(P * j, P * (j + 1))
            gcols = [(0, 512), (512, 1024), (1024, 1280)]
            if final:
                # sequential groups with per-group writeback: only the last
                # (smallest) group's copy+DMA trail the final matmul
                for g in range(3):
                    c0, c1 = gcols[g]
                    pg = main_psum.tile([P, 512], F32, name=f"pg{j}_{g}",
                                        tag="mp")
                    psl = pg[:, 0:c1 - c0]
                    for q in range(8):
                        nc.tensor.matmul(psl, lhsT=z[0][q][:, bsl],
                                         rhs=R0_sb[:, q, c0:c1],
                                         start=(q == 0), stop=False)
                    if not skip_lb2:
                        nc.tensor.matmul(psl, lhsT=xbc[0][0:16, bsl],
                                         rhs=R0x_sb[:, c0:c1],
                                         start=False, stop=False)
                    nc.tensor.matmul(psl, lhsT=hbp_sb[:, bsl],
                                     rhs=BB_sb[:, c0:c1],
                                     start=False, stop=True)
                    copy_out(P1_ENG[j][g], out_t[j][:, c0:c1], psl)
                    nc.sync.dma_start(t["out"][bsl, c0:c1],
                                      out_t[j][:, c0:c1])
                return
            pg = [main_psum.tile([P, 512], F32, name=f"pg{j}_{g}", tag="mp")
                  for g in range(3)]
            gsl = [pg[0][:], pg[1][:], pg[2][:, 0:256]]
            for q in range(8):
                if j == 0:
                    z_mul(0, q)
                for g in range(3):
                    c0, c1 = gcols[g]
                    nc.tensor.matmul(gsl[g], lhsT=z[0][q][:, bsl],
                                     rhs=R0_sb[:, q, c0:c1],
                                     start=(q == 0), stop=False)
                if j == 0:
                    # the next replication matmul comes AFTER this q's main
                    # matmuls so a late weight DMA can't block them in-order
                    if q < 6:
                        h_mm(q + 2)
                    else:
                        x_mm(1 if q == 6 else 2)
            if j == 0:
                x_mm(3)
                for tdx in range(1, 4):
                    for q in range(8):
                        z_mul(tdx, q)
            if not skip_lb2:
                for g in range(3):
                    c0, c1 = gcols[g]
                    nc.tensor.matmul(gsl[g], lhsT=xbc[0][0:16, bsl],
                                     rhs=R0x_sb[:, c0:c1],
                                     start=False, stop=False)
            for g in range(3):
                c0, c1 = gcols[g]
                nc.tensor.matmul(gsl[g], lhsT=hbp_sb[:, bsl],
                                 rhs=BB_sb[:, c0:c1], start=False, stop=True)
            for g in range(3):
                c0, c1 = gcols[g]
                copy_out(P1_ENG[j][g], out_t[j][:, c0:c1], gsl[g])
            nc.sync.dma_start(t["out"][bsl, 0:1280], out_t[j][:, 0:1280])

        for j in range(NB - 1):
            phase1(j, final=False)

        for phase in range(2):
            rc = (0, 512) if phase == 0 else (512, 1024)
            for j in range(NB):
                bsl = slice(P * j, P * (j + 1))
                for mi in range(3):
                    m = 3 * phase + mi
                    tdx = 1 + mi
                    pgm = main_psum.tile([P, 512], F32, name=f"pm{j}_{m}",
                                         tag="mp")
                    for q in range(8):
                        last = skip_lb2 and q == 7
                        nc.tensor.matmul(pgm[:], lhsT=z[tdx][q][:, bsl],
                                         rhs=R1_sb[:, q, rc[0]:rc[1]],
                                         start=(q == 0), stop=last)
                    if not skip_lb2:
                        nc.tensor.matmul(pgm[:], lhsT=xbc[tdx][0:16, bsl],
                                         rhs=R1x_sb[:, rc[0]:rc[1]],
                                         start=False, stop=True)
                    c0 = 1280 + 512 * m
                    copy_out(P23_ENG[j][m], out_t[j][:, c0:c0 + 512], pgm[:])
                c0 = 1280 + 1536 * phase
                nc.sync.dma_start(t["out"][bsl, c0:c0 + 1536],
                                  out_t[j][:, c0:c0 + 1536])

        phase1(NB - 1, final=True)


def _prepare(inputs, mode):
    f32 = np.float32
    feat = np.ascontiguousarray(np.asarray(inputs["feat"], dtype=f32))
    node_emb = np.ascontiguousarray(np.asarray(inputs["node_emb"], dtype=f32))
    W0 = np.asarray(inputs["W0"], f32)
    W1 = np.asarray(inputs["W1"], f32)
    lw1 = np.asarray(inputs["lw1"], f32)
    lb1 = np.asarray(inputs["lb1"], f32)
    lw2 = np.asarray(inputs["lw2"], f32)
    lb2 = np.asarray(inputs["lb2"], f32)
    bw1 = np.asarray(inputs["bw1"], f32)
    bb1 = np.asarray(inputs["bb1"], f32)
    bw2 = np.asarray(inputs["bw2"], f32)
    bb2 = np.asarray(inputs["bb2"], f32)

    mmnp = _np_mm_dtype(mode)
    s16 = np.float32(1.0 / 16.0)
    sC = np.float32(C3 / 16.0)

    # weight matrices for the main contraction, path scales folded in
    lw2p = np.concatenate([lw2, lb2[None]], axis=0)           # [65, 36864]
    M00 = lw2p[:, :16384].reshape(1040, 1024) * s16
    M11 = lw2p[:, 16384:20480].reshape(1040, 256) * sC
    M01 = lw2p[:, 20480:28672].reshape(1040, 512) * sC
    M10 = lw2p[:, 28672:36864].reshape(1040, 512) * sC
    R0f = np.concatenate([M00, M11], axis=1)                  # [1040, 1280]
    R1f = np.concatenate([M01, M10], axis=1)                  # [1040, 1024]
    R0 = np.ascontiguousarray(R0f[0:1024]).astype(mmnp)
    R1 = np.ascontiguousarray(R1f[0:1024]).astype(mmnp)
    R0x = np.ascontiguousarray(R0f[1024:1040]).astype(mmnp)
    R1x = np.ascontiguousarray(R1f[1024:1040]).astype(mmnp)
    BBf = np.concatenate([bw2, bb2[None]], axis=0)            # [65, 1280]
    BBp = np.ascontiguousarray(
        np.concatenate([BBf[:, :1024] * s16, BBf[:, 1024:] * sC], axis=1)
    ).astype(mmnp)

    # replicated-column weights: output partition (c8,w) = 16*c8 + w
    W0s = W0 * np.float32(1.0 / np.sqrt(128.0))               # [128, 16]
    W1s = W1 * np.float32(1.0 / 8.0)                          # [64, 16]
    rep = np.arange(1024)
    gsel = (rep // 128) * 8 + (rep % 128) // 16               # c = 8q + c8
    lw1rep = lw1[:, gsel]                                     # [128, 1024]
    W0rep = np.tile(W0s, (1, 8))                              # [128, 128]
    W1rep = np.tile(W1s, (1, 8))                              # [64, 128]
    # layout must match HCOL & friends in _emit (embT in cols 0:512,
    # filled per core below)
    wpk = np.zeros((128, 1872), f32)
    hcol = [512, 640, 976, 1104, 1232, 1360, 1488, 1616]
    for q in range(8):
        wpk[:, hcol[q]:hcol[q] + 128] = lw1rep[:, 128 * q:128 * (q + 1)]
        wpk[:, 768 + q] = lb1[8 * q + np.arange(128) // 16]
    wpk[0:64, 776] = bb1
    wpk[:, 784:848] = bw1
    wpk[:, 848:976] = W0rep
    wpk[0:64, 1744:1872] = W1rep
    wpk = wpk.astype(mmnp)

    skip_lb2 = not bool(np.any(lb2))

    in_maps = []
    for i in range(N_CORES):
        sl = slice(i * BC, (i + 1) * BC)
        fs = feat[sl]
        featT = np.ascontiguousarray(
            np.concatenate(
                [fs[:, :128], fs[:, 128::3], fs[:, 129::3], fs[:, 130::3]],
                axis=1).T.astype(mmnp))                       # [320, BC]
        wpk_i = wpk.copy()
        wpk_i[:, 0:512] = node_emb[sl].T.astype(mmnp)
        m = {
            "featT": featT,
            "wpk": np.ascontiguousarray(wpk_i), "BBp": BBp,
            "R0": R0, "R1": R1,
        }
        if not skip_lb2:
            m["R0x"] = R0x
            m["R1x"] = R1x
        in_maps.append(m)
    return in_maps, skip_lb2


def _unpack_output(buf):
    """[B, 4352] packed columns -> [B, 80, 80] float32."""
    bf = buf.astype(np.float32)
    n = bf.shape[0]
    out3 = np.zeros((n, 80, 80), np.float32)
    out3[:, 0:16, 0:32] = bf[:, 0:512].reshape(n, 16, 32)
    out3[:, 16:32, 0:32] = bf[:, 512:1024].reshape(n, 16, 32)
    p11 = bf[:, 1024:1280].reshape(n, 16, 16)
    for i in range(3):
        out3[:, 32 + i::3, 32 + i::3] = p11
    for k in range(3):
        out3[:, 0:32, 32 + k::3] = \
            bf[:, 1280 + 512 * k:1792 + 512 * k].reshape(n, 32, 16)
    for i in range(3):
        out3[:, 32 + i::3, 0:32] = \
            bf[:, 2816 + 512 * i:3328 + 512 * i].reshape(n, 16, 32)
    return out3


def run(inputs, mode=None, trace=False):
    """Build (cached), run on 8 cores, gather. Returns (out, results)."""
    mode = mode or MM_MODE
    in_maps, skip_lb2 = _prepare(inputs, mode)
    key = (mode, skip_lb2)
    if key not in _CACHE:
        _CACHE[key] = _build_program(mode, skip_lb2)
    nc = _CACHE[key]

    from concourse.bass_utils import run_bass_kernel_spmd
    res = run_bass_kernel_spmd(nc, in_maps, list(range(N_CORES)), trace=trace)
    buf = np.concatenate([res.results[i]["out"] for i in range(N_CORES)], axis=0)
    return _unpack_output(buf), res


def kernel(**inputs):
    out, _ = run(inputs)
    return out
